# revision 9
# baseline (speedup 1.0000x reference)
"""Trainium2 Bass kernel for the spectral history-filter model (nn_DSC_23252952941334).

Math: all four reference terms are linear in y_hist with per-lag effective
weight matrices, so the whole module collapses to

    out[b, c] = sum_{j=0..63} sum_p  W_time[j][c, p] * y_hist[b, j+2, p]

where W_time[j] combines M_bar_0 / M_bar / M_0l / M_il with the small
spectral coefficient vectors (folded on host in float64 — ~5 MFLOP).

Device strategy (8 cores, data-parallel over batch):
  - host: fold weights, transpose each core's y shard to [k, b] layout
    (k = (lag j, p) on SBUF partitions; contraction dim must be the
    partition dim for the PE array)
  - device: out[c, b] = sum_k w[k, c] * y[k, b] as 64 accumulated
    128-contraction matmuls into 4 PSUM banks (b free dim 512 each)
  - host: gather per-core [c, b] outputs, transpose to (B, 128)
"""

import os
import numpy as np
from concurrent.futures import ThreadPoolExecutor

N_CORES = 8
B, L, P, MC = 16384, 66, 128, 128
H, M = 16, 32
NLAG = 64            # lags d=0..63 <-> y time indices 2..65
BS = B // N_CORES    # 2048 batch rows per core
NBT = 4              # psum b-tiles of 512
KJ = 2               # k-chunks (128 rows each) per DMA super-chunk

# float32r runs the PE at 1 cycle/row (vs 4 for float32) at free dim >= 256.
USE_F32R = os.environ.get("KERNEL_F32", "0") != "1"
TRACE = False        # test.py can flip this to get a profile

_cached_nc = {}


def _fold_weights(M_bar_0, M_bar, M_0l, M_il, sigma_powered, phi,
                  lambda_powered, varphi):
    """Return w_dev (P, NLAG, MC) fp32 with w_dev[p, j, c] = W_time[j][c, p]."""
    f8 = np.float64
    M_bar_0 = M_bar_0.astype(f8); M_bar = M_bar.astype(f8)
    M_0l = M_0l.astype(f8); M_il = M_il.astype(f8)
    sig = sigma_powered.astype(f8); lam = lambda_powered.astype(f8)
    phi = phi.astype(f8); varphi = varphi.astype(f8)

    # W_lag[d] acts on Yr[:, d] = y[:, L-1-d]
    W = np.zeros((NLAG, MC, P), f8)
    W[0] = M_bar_0

    # term 2: sum_i lam[i] * varphi[j, i] * M_bar[i] on lag j+1
    coef2 = varphi @ np.diag(lam)                       # (M, H) -> [j, i]
    W[1:M + 1] += np.einsum('ji,icp->jcp', coef2, M_bar)

    # term 3: sum_l sigma_ext[l] * phi_ext[k, l] * M_0l[l] on lag k+1
    sigma_ext = np.concatenate([[1.0], sig])            # (H+1,)
    phi_ext = np.concatenate([np.ones((M, 1)), phi], 1)  # (M, H+1)
    coef3 = phi_ext @ np.diag(sigma_ext)                # (M, H+1) -> [k, l]
    W[1:M + 1] += np.einsum('kl,lcp->kcp', coef3, M_0l)

    # term 4: anti-diagonal fold of varphi[j,i] phi_ext[k,l] comb[l,i] M_il[i,l]
    comb = sigma_ext[:, None] * lam[None, :]            # (H+1, H) -> [l, i]
    corr = np.zeros((2 * M - 1, H + 1, H), f8)          # [d, l, i]
    for j in range(M):
        for k in range(M):
            corr[j + k] += phi_ext[k][:, None] * varphi[j][None, :]
    C4 = corr * comb[None]                              # (2M-1, H+1, H)
    W[1:2 * M] += np.einsum('dli,ilcp->dcp', C4, M_il)

    # reorder to ascending time index: W_time[j] = W_lag[63 - j]
    # and lay out for SBUF: w_dev[p, j, c]
    w_dev = np.ascontiguousarray(W[::-1].transpose(2, 0, 1)).astype(np.float32)
    return w_dev


def _transpose_shards(y):
    """y (B, L, P) fp32 -> list of per-core yt (NLAG, P, BS) fp32,
    yt[j, p, b] = y[core*BS + b, j + 2, p]."""
    src = y[:, 2:, :]                  # (B, 64, 128) strided view
    shards = [np.empty((NLAG, P, BS), np.float32) for _ in range(N_CORES)]
    BB = 128

    def work(args):
        ci, b0 = args
        blk = np.ascontiguousarray(src[ci * BS + b0: ci * BS + b0 + BB])
        shards[ci][:, :, b0:b0 + BB] = blk.transpose(1, 2, 0)

    jobs = [(ci, b0) for ci in range(N_CORES) for b0 in range(0, BS, BB)]
    with ThreadPoolExecutor(8) as ex:
        list(ex.map(work, jobs))
    return shards


def _build_nc(use_f32r):
    import concourse.bass as bass
    import concourse.tile as tile
    from concourse import mybir
    from concourse.bass import ts
    from contextlib import ExitStack

    DT = mybir.dt.float32
    RDT = mybir.dt.float32r if use_f32r else mybir.dt.float32

    nc = bass.Bass()
    yt = nc.declare_dram_parameter("yt", [NLAG, P, BS], DT, isOutput=False)
    w = nc.declare_dram_parameter("w", [P, NLAG, MC], DT, isOutput=False)
    out = nc.declare_dram_parameter("out", [MC, BS], DT, isOutput=True)

    NSUP = NLAG // KJ  # 16 DMA super-chunks

    with ExitStack() as ctx:
        tc = ctx.enter_context(tile.TileContext(nc))
        wpool = ctx.enter_context(tc.tile_pool(name="wp", bufs=1))
        ypool = ctx.enter_context(tc.tile_pool(name="yp", bufs=8))
        opool = ctx.enter_context(tc.tile_pool(name="op", bufs=1))
        pspool = ctx.enter_context(tc.tile_pool(name="ps", bufs=1, space="PSUM"))

        wtile = wpool.tile([P, NLAG, MC], RDT)
        nc.sync.dma_start(wtile[:], w[:].bitcast(RDT))

        psums = [pspool.tile([MC, 512], mybir.dt.float32, name=f"ps{t}")
                 for t in range(NBT)]

        # Warm-up matmul consuming wtile: gives the PE a single-wait
        # observation of the w-DMA (walrus rejects >1 sync wait on a
        # matmul) and starts the HAM clock ramp before the real work.
        warm = pspool.tile([MC, MC], mybir.dt.float32, name="warm")
        nc.tensor.matmul(warm[:], wtile[:, 0, :], wtile[:, 0, :],
                         start=True, stop=True)

        for kk in range(NSUP):
            ytile = ypool.tile([P, KJ, BS], RDT)
            src = yt[kk * KJ:(kk + 1) * KJ, :, :].rearrange("j p b -> p j b")
            nc.sync.dma_start(ytile[:], src.bitcast(RDT))
            for jj in range(KJ):
                j = kk * KJ + jj
                lhsT = wtile[:, j, :]
                for t in range(NBT):
                    rhs = ytile[:, jj, ts(t, 512)]
                    nc.tensor.matmul(psums[t][:], lhsT, rhs,
                                     start=(j == 0), stop=(j == NLAG - 1))

        outt = opool.tile([MC, BS], DT)
        for t in range(NBT):
            nc.vector.tensor_copy(outt[:, ts(t, 512)], psums[t][:])
        nc.sync.dma_start(out[:], outt[:])

    return nc


def _strip_redundant_waits(nc):
    """Drop semaphore waits that are provably implied by other waits.

    Tile's add_semaphores pass is per-processor minimal but not transitively
    minimal; walrus codegen allows only one sync wait per DMA/Matmult/Drain
    HW instruction.  Model:
      - expand(s >= v) = {s >= v} union C[sat(s, v)] where sat is the
        instruction whose semaphore update first reaches v (updates on one
        engine sem / one DMA lane fire in order).
      - C[i] ("true once i's updates fired") = own updates + expand(own
        waits) + dispatch-knowledge (expand of same-engine predecessors'
        waits; sequencers evaluate waits in program order) + C[predecessor]
        chained in completion order: same engine for compute engines, same
        DMA lane for DMACopy (async transfers complete in ring order).
    A wait w on i is droppable iff w is in (expand of same-engine
    predecessors' waits) union (expand of i's other waits).
    """
    from concourse import mybir

    f = nc.m.functions[0]
    insts = [i for blk in f.blocks for i in blk.instructions]

    def waits(i):
        si = i.sync_info
        return [(w.ant_name, w.wait_value) for w in (si.on_wait or [])] \
            if si else []

    def updates(i):
        si = i.sync_info
        return list(si.on_update or []) if si else []

    by_engine = {}
    for i in insts:
        by_engine.setdefault(str(i.engine), []).append(i)

    COMPUTE = {"EngineType.PE", "EngineType.DVE", "EngineType.Activation",
               "EngineType.Pool"}

    # cumulative update values per sem, in program order of the updater
    sem_updates = {}           # sem -> [(inst_name, cumulative)]
    upd_of = {i.name: [] for i in insts}
    for eng, lst in by_engine.items():
        for i in lst:
            for u in updates(i):
                cum = sem_updates.setdefault(u.ant_name, [])
                prev = cum[-1][1] if cum else 0
                val = u.update_value if u.update_mode == "sem-add-imm" else 1
                cum.append((i.name, prev + val))
                upd_of[i.name].append((u.ant_name, prev + val))

    def satisfier(sem, v):
        for name, val in sem_updates.get(sem, ()):
            if val >= v:
                return name
        return None

    # completion-order predecessor: same engine (compute) or same DMA lane
    comp_pred = {}
    last_on_lane = {}
    for eng, lst in by_engine.items():
        prev = None
        for i in lst:
            if type(i).__name__ == "InstDMACopy":
                lanes = [s for s, _ in upd_of[i.name]]
                lane = lanes[0] if lanes else None
                comp_pred[i.name] = last_on_lane.get(lane)
                if lane is not None:
                    last_on_lane[lane] = i.name
            elif eng in COMPUTE:
                comp_pred[i.name] = prev
            else:
                comp_pred[i.name] = None
            prev = i.name

    # dispatch-order predecessor (same engine, any type)
    disp_pred = {}
    for eng, lst in by_engine.items():
        prev = None
        for i in lst:
            disp_pred[i.name] = prev
            prev = i.name

    C = {i.name: {} for i in insts}
    DW = {i.name: {} for i in insts}   # dispatch knowledge (pred waits, cum.)
    name2inst = {i.name: i for i in insts}

    def merge(dst, src_items):
        ch = False
        for s, v in src_items:
            if dst.get(s, 0) < v:
                dst[s] = v
                ch = True
        return ch

    changed = True
    rounds = 0
    while changed and rounds < 100:
        changed = False
        rounds += 1
        for i in insts:
            n = i.name
            # DW: dispatch knowledge = pred's DW + expand(pred's waits)
            dp = disp_pred[n]
            if dp is not None:
                changed |= merge(DW[n], DW[dp].items())
                for s, v in waits(name2inst[dp]):
                    changed |= merge(DW[n], [(s, v)])
                    j = satisfier(s, v)
                    if j is not None:
                        changed |= merge(DW[n], C[j].items())
            # C: completion closure
            changed |= merge(C[n], DW[n].items())
            changed |= merge(C[n], upd_of[n])
            cp = comp_pred.get(n)
            if cp is not None:
                changed |= merge(C[n], C[cp].items())
            for s, v in waits(i):
                changed |= merge(C[n], [(s, v)])
                j = satisfier(s, v)
                if j is not None:
                    changed |= merge(C[n], C[j].items())

    for i in insts:
        si = i.sync_info
        if not si or len(si.on_wait or []) <= 1:
            continue
        if type(i).__name__ not in ("InstDMACopy", "InstMatmult",
                                    "InstDrain"):
            continue
        keep = []
        for w in si.on_wait:
            avail = dict(DW[i.name])
            for w2 in si.on_wait:
                if w2 is w:
                    continue
                merge(avail, [(w2.ant_name, w2.wait_value)])
                j = satisfier(w2.ant_name, w2.wait_value)
                if j is not None:
                    merge(avail, C[j].items())
            if avail.get(w.ant_name, 0) < w.wait_value:
                keep.append(w)
        if len(keep) > 1:
            raise RuntimeError(
                f"{i.name}: still {len(keep)} waits after stripping: "
                f"{[(w.ant_name, w.wait_value) for w in keep]}")
        if len(keep) != len(si.on_wait):
            i.sync_info = mybir.SyncInfo(
                on_wait=keep, on_update=list(si.on_update or []))
    return nc


def _get_nc(use_f32r):
    key = bool(use_f32r)
    if key not in _cached_nc:
        _cached_nc[key] = _strip_redundant_waits(_build_nc(key))
    return _cached_nc[key]


def kernel(y_hist, M_bar_0, M_bar, M_0l, M_il, sigma_powered, phi,
           lambda_powered, varphi):
    from concourse.bass_utils import run_bass_kernel_spmd

    y_hist = np.ascontiguousarray(np.asarray(y_hist, dtype=np.float32))
    w_dev = _fold_weights(np.asarray(M_bar_0), np.asarray(M_bar),
                          np.asarray(M_0l), np.asarray(M_il),
                          np.asarray(sigma_powered), np.asarray(phi),
                          np.asarray(lambda_powered), np.asarray(varphi))
    shards = _transpose_shards(y_hist)

    nc = _get_nc(USE_F32R)
    in_maps = [{"yt": shards[ci], "w": w_dev} for ci in range(N_CORES)]
    res = run_bass_kernel_spmd(nc, in_maps, list(range(N_CORES)), trace=TRACE)

    if TRACE:
        kernel.last_result = res

    out = np.empty((B, MC), np.float32)
    for ci in range(N_CORES):
        out[ci * BS:(ci + 1) * BS] = res.results[ci]["out"].T
    return out
